# revision 1
# baseline (speedup 1.0000x reference)
"""Tensor-parallel Llama attention (decode, GQA, RoPE, KV-cache) on 8 TRN2 cores.

Sharding: core c owns kv-head c and q-heads 4c..4c+3. Wq/Wk/Wv are sharded
column-wise, Wo row-wise; each core computes a partial o_proj output and the
host sums the 8 partials (the all-reduce).

Per-core kernel layout notes:
  - Everything is kept "transposed" ([d, token] / [d, kpos]) so that every
    matmul contracts over the partition dim with M=128 (full PE array):
      qT/kT/vnew from projections, scoresT = kT_tile.T @ qT, attnT = v.T @ exp.
  - Softmax runs without max-subtraction (|score| <= ~8 here, exp is safe in
    fp32) so the kpos-partition layout only needs a sum: DVE accumulates exp
    tiles, a ones-column matmul reduces over partitions, and a 1x128 ones
    matmul broadcasts 1/denom back over partitions.
  - The causal mask only affects the 16 fresh keys (bottom-right aligned),
    applied as a 0/1 multiply on the one small fresh-score tile.
"""

import numpy as np
import ml_dtypes

import concourse.bass as bass
import concourse.mybir as mybir
import concourse.tile as tile
from concourse import bacc
from concourse.bass_utils import run_bass_kernel_spmd

F32 = mybir.dt.float32
BF16 = mybir.dt.bfloat16
AF = mybir.ActivationFunctionType

# Problem shape (hardcoded per contract)
B, S, H = 4, 16, 4096
NH, NKV, HD = 32, 8, 128
PAST = 8192
ROPE_BASE = 10000.0
NCORES = 8
HQ = NH // NCORES          # q heads per core = 4
TOK = B * S                # 64 tokens
NCH = H // 128             # 32 contraction chunks for projections
ROWS = HQ * S              # 64 (head, token) query rows per batch
SCALE = HD ** -0.5


def build_nc(b=B, s=S, h=H, hq=HQ, hd=HD, past=PAST):
    tok = b * s
    nch = h // 128
    rows = hq * s
    ktiles = past // 128
    halves = 2                      # stream k/v caches in 2 chunks per batch
    kt_half = ktiles // halves

    nc = bacc.Bacc("TRN2", target_bir_lowering=False, debug=False)

    hiddenT_d = nc.dram_tensor("hiddenT", [h, tok], BF16, kind="ExternalInput").ap()
    wq_d = nc.dram_tensor("wq", [h, hq * hd], BF16, kind="ExternalInput").ap()
    wkv_d = nc.dram_tensor("wkv", [h, 2 * hd], BF16, kind="ExternalInput").ap()
    wo_d = nc.dram_tensor("wo", [hq * hd, h], BF16, kind="ExternalInput").ap()
    kT_d = nc.dram_tensor("kT", [b, hd, past], BF16, kind="ExternalInput").ap()
    v_d = nc.dram_tensor("v", [b, 128, past], BF16, kind="ExternalInput").ap()
    cosT_d = nc.dram_tensor("cosT", [hd, tok], F32, kind="ExternalInput").ap()
    sinT_d = nc.dram_tensor("sinT", [hd, tok], F32, kind="ExternalInput").ap()
    nsinT_d = nc.dram_tensor("nsinT", [hd, tok], F32, kind="ExternalInput").ap()
    maskT_d = nc.dram_tensor("maskT", [s, rows], F32, kind="ExternalInput").ap()
    out_d = nc.dram_tensor("out_p", [tok, h], F32, kind="ExternalOutput").ap()

    with tile.TileContext(nc) as tc:
        import contextlib

        with contextlib.ExitStack() as ctx:
            ep = ctx.enter_context          # shorthand
            const_p = ep(tc.tile_pool(name="const", bufs=1))
            hT_p = ep(tc.tile_pool(name="hT", bufs=1))
            wq_p = ep(tc.tile_pool(name="wq", bufs=3))
            wkv_p = ep(tc.tile_pool(name="wkv", bufs=3))
            wo_p = ep(tc.tile_pool(name="wo", bufs=32))
            kv_p = ep(tc.tile_pool(name="kv", bufs=6))
            qkv_p = ep(tc.tile_pool(name="qkv", bufs=1))
            rope_p = ep(tc.tile_pool(name="rope", bufs=4))
            exp_p = ep(tc.tile_pool(name="exp", bufs=6))
            acc_p = ep(tc.tile_pool(name="acc", bufs=2))
            den_p = ep(tc.tile_pool(name="den", bufs=2))
            # PSUM: 8 banks total; tags share banks across phases:
            #   "A"(2): qt (proj) -> ops (o_proj);  "attn"(2): per-batch attn acc
            #   "B"(2): ktn+vn (proj) -> dsum/bc (softmax);  "sc"(2): score tiles
            ps = ep(tc.tile_pool(name="ps", bufs=2, space="PSUM"))

            # ---- constants ----
            ones_col = const_p.tile([128, 1], F32)
            nc.vector.memset(ones_col[:], 1.0)
            ones_row = const_p.tile([1, 128], F32)
            nc.vector.memset(ones_row[:], 1.0)
            cosT = const_p.tile([hd, tok], F32)
            nc.sync.dma_start(cosT[:], cosT_d[:])
            sinT = const_p.tile([hd, tok], F32)
            nc.sync.dma_start(sinT[:], sinT_d[:])
            nsinT = const_p.tile([hd, tok], F32)
            nc.sync.dma_start(nsinT[:], nsinT_d[:])
            maskT = const_p.tile([s, rows], F32)
            nc.sync.dma_start(maskT[:], maskT_d[:])
            ident = const_p.tile([tok, tok], F32)
            from concourse.masks import make_identity
            make_identity(nc, ident[:])

            # ---- load hiddenT: [h, tok] -> sbuf [128, nch*tok] ----
            hT = hT_p.tile([128, nch * tok], BF16)
            nc.sync.dma_start(
                hT[:].rearrange("p (c t) -> p c t", c=nch),
                hiddenT_d.rearrange("(c p) t -> p c t", p=128),
            )

            # ---- projections: qT_ps[j] [128, tok], kT_ps [128, tok], v_ps [tok, 128] ----
            # q in token-major [tok, hq*hd] (single PSUM bank/group); k/v direct
            q_ps = ps.tile([tok, hq * hd], F32, tag="A")
            kT_ps = ps.tile([128, tok], F32, tag="B")
            v_ps = ps.tile([tok, 128], F32, tag="B")
            for c in range(nch):
                wq_t = wq_p.tile([128, hq * hd], BF16)
                nc.sync.dma_start(
                    wq_t[:], wq_d.rearrange("(c p) m -> c p m", p=128)[c]
                )
                wkv_t = wkv_p.tile([128, 2 * hd], BF16)
                nc.sync.dma_start(
                    wkv_t[:], wkv_d.rearrange("(c p) m -> c p m", p=128)[c]
                )
                rhs_h = hT[:, c * tok:(c + 1) * tok]
                fl = dict(start=(c == 0), stop=(c == nch - 1))
                nc.tensor.matmul(q_ps[:], rhs_h, wq_t[:], **fl)
                nc.tensor.matmul(kT_ps[:], wkv_t[:, 0:hd], rhs_h, **fl)
                nc.tensor.matmul(v_ps[:], rhs_h, wkv_t[:, hd:2 * hd], **fl)
            q_sb = qkv_p.tile([tok, hq * hd], F32, tag="qsb")
            nc.scalar.copy(q_sb[:], q_ps[:])

            # ---- RoPE -> qT_sb [128, (b,hq,s)], kT_new [128, (b,s)], v_new [tok, 128] ----
            half = hd // 2
            qT_sb = qkv_p.tile([128, b * rows], F32, tag="qT")
            kT_new = qkv_p.tile([128, tok], F32, tag="kTn")
            # per-batch fresh-v tiles at base partition 0 (PE wants base 0/32/64)
            v_new = [
                qkv_p.tile([s, hd], F32, tag=f"vnew{bb}", name=f"vnew{bb}")
                for bb in range(b)
            ]

            def rope(dst, src_ps):
                # dst = src*cos + rotate_half(src)*sin  (all [128, tok], (b,t) cols)
                t1 = rope_p.tile([128, tok], F32, tag="r1")
                nc.vector.tensor_mul(t1[:], src_ps[:], cosT[:])
                t2 = rope_p.tile([128, tok], F32, tag="r2")
                nc.vector.tensor_mul(
                    t2[0:half, :], src_ps[half:hd, :], nsinT[0:half, :]
                )
                nc.vector.tensor_mul(
                    t2[half:hd, :], src_ps[0:half, :], sinT[half:hd, :]
                )
                nc.vector.tensor_add(dst, t1[:], t2[:])
                return dst

            for j in range(hq):
                # transpose head j to [d, (b,t)], then rope-scatter to (b, j, t)
                qt_ps = ps.tile([hd, tok], F32, tag="sc", name=f"qtp{j}")
                nc.tensor.transpose(
                    qt_ps[:], q_sb[:, j * hd:(j + 1) * hd], ident[:]
                )
                dst = qT_sb[:].rearrange("p (bb j t) -> p bb j t", bb=b, j=hq)[:, :, j, :]
                rope(dst, qt_ps)
            rope(kT_new[:], kT_ps)
            v_sb = qkv_p.tile([tok, hd], F32, tag="vsb")
            nc.scalar.copy(v_sb[:], v_ps[:])
            for bb in range(b):
                nc.sync.dma_start(v_new[bb][:], v_sb[bb * s:(bb + 1) * s, :])

            qT_bf = qkv_p.tile([128, b * rows], BF16, tag="qTbf")
            nc.vector.tensor_copy(qT_bf[:], qT_sb[:])

            # ---- attention per batch ----
            # Scores are built 8 kpos-tiles at a time into ONE psum bank
            # (disjoint column ranges, one accumulation group) so exp / the
            # denominator reduce run 512 wide, 8x fewer cross-engine hops.
            GRP = 512 // rows               # kpos tiles per score group (8)
            attnT_sb = qkv_p.tile([128, hq * tok], BF16, tag="attnT")  # (h, b, t) cols
            for bb in range(b):
                qT_b = qT_bf[:, bb * rows:(bb + 1) * rows]  # [128, (h,t)] bf16
                qT_b32 = qT_sb[:, bb * rows:(bb + 1) * rows]
                attn_ps = ps.tile([128, rows], F32, tag="attn")
                acc = acc_p.tile([128, rows], F32, tag="acc")
                for hf in range(halves):
                    kt = kv_p.tile([128, kt_half * 128], BF16, tag="kt")
                    nc.sync.dma_start(
                        kt[:], kT_d[bb, :, hf * kt_half * 128:(hf + 1) * kt_half * 128]
                    )
                    vt = kv_p.tile([128, kt_half * hd], BF16, tag="vt")
                    nc.sync.dma_start(
                        vt[:],
                        v_d[bb, :, hf * kt_half * hd:(hf + 1) * kt_half * hd],
                    )
                    for g in range(kt_half // GRP):
                        sc_ps = ps.tile([128, GRP * rows], F32, tag="sc")
                        for u in range(GRP):
                            tt = g * GRP + u
                            nc.tensor.matmul(
                                sc_ps[:, u * rows:(u + 1) * rows],
                                kt[:, tt * 128:(tt + 1) * 128], qT_b,
                                start=(u == 0), stop=(u == GRP - 1),
                            )
                        ex = exp_p.tile([128, GRP * rows], BF16, tag="ex")
                        nc.scalar.activation(ex[:], sc_ps[:], AF.Exp)
                        red = acc if (hf == 0 and g == 0) else acc_p.tile(
                            [128, rows], F32, tag="red", name="red")
                        nc.vector.tensor_reduce(
                            red[:],
                            ex[:].rearrange("p (u q) -> p q u", u=GRP),
                            axis=mybir.AxisListType.X, op=mybir.AluOpType.add,
                        )
                        if red is not acc:
                            nc.vector.tensor_add(acc[:], acc[:], red[:])
                        for u in range(GRP):
                            tt = g * GRP + u
                            t = hf * kt_half + tt
                            nc.tensor.matmul(
                                attn_ps[:], vt[:, tt * hd:(tt + 1) * hd],
                                ex[:, u * rows:(u + 1) * rows],
                                start=(t == 0), stop=False, skip_group_check=True,
                            )
                # fresh keys (the only masked block)
                scn_ps = ps.tile([s, rows], F32, tag="sc")
                nc.tensor.matmul(
                    scn_ps[:], kT_new[:, bb * s:(bb + 1) * s], qT_b32,
                    start=True, stop=True,
                )
                exn = exp_p.tile([s, rows], F32, tag="exn")
                nc.scalar.activation(exn[:], scn_ps[:], AF.Exp)
                nc.vector.tensor_mul(exn[:], exn[:], maskT[:])
                nc.vector.tensor_add(acc[0:s, :], acc[0:s, :], exn[:])
                nc.tensor.matmul(
                    attn_ps[:], v_new[bb][:], exn[:],
                    start=False, stop=True, skip_group_check=True,
                )
                # denominator: reduce acc over partitions, broadcast reciprocal
                dsum_ps = ps.tile([1, rows], F32, tag="B")
                nc.tensor.matmul(dsum_ps[:], ones_col[:], acc[:], start=True, stop=True)
                rden = den_p.tile([1, rows], F32, tag="rden")
                nc.vector.reciprocal(rden[:], dsum_ps[:])
                bc_ps = ps.tile([128, rows], F32, tag="B")
                nc.tensor.matmul(bc_ps[:], ones_row[:], rden[:], start=True, stop=True)
                rdenb = den_p.tile([128, rows], F32, tag="rdenb")
                nc.scalar.copy(rdenb[:], bc_ps[:])
                # normalize + scatter (h,t) -> (h, b, t)
                dst = attnT_sb[:].rearrange("p (j bb t) -> p j bb t", j=hq, bb=b)[
                    :, :, bb, :
                ]
                nc.vector.tensor_mul(
                    dst,
                    attn_ps[:].rearrange("p (j t) -> p j t", j=hq),
                    rdenb[:].rearrange("p (j t) -> p j t", j=hq),
                )

            # ---- o_proj: out[tok, h] = sum_j attnT_j.T @ wo_j ----
            for nt in range(h // 512):
                o_ps = ps.tile([tok, 512], F32, tag="A")
                for j in range(hq):
                    wo_t = wo_p.tile([128, 512], BF16, tag="wo")
                    nc.sync.dma_start(
                        wo_t[:],
                        wo_d.rearrange("(j p) m -> j p m", p=128)[
                            j, :, nt * 512:(nt + 1) * 512
                        ],
                    )
                    nc.tensor.matmul(
                        o_ps[:], attnT_sb[:, j * tok:(j + 1) * tok], wo_t[:],
                        start=(j == 0), stop=(j == hq - 1),
                    )
                o_sb = wo_p.tile([tok, 512], F32, tag="osb", bufs=3)
                nc.scalar.copy(o_sb[:], o_ps[:])
                nc.sync.dma_start(out_d[:, nt * 512:(nt + 1) * 512], o_sb[:])

    nc.compile()
    return nc


_NC_CACHE = {}


def _get_nc(key=(B, S, H, HQ, HD, PAST)):
    if key not in _NC_CACHE:
        _NC_CACHE[key] = build_nc(*key)
    return _NC_CACHE[key]


def make_in_maps(hidden_states, k_cache, v_cache, Wq, Wk, Wv, Wo, position_ids):
    """Host-side shard + layout prep: one input dict per core."""
    hiddenT = np.ascontiguousarray(
        hidden_states.reshape(TOK, H).T.astype(np.float32)
    ).astype(ml_dtypes.bfloat16)
    # RoPE tables in [d, (b, t)] layout, duplicated freq block (half-split rope)
    inv_freq = (1.0 / (ROPE_BASE ** (np.arange(0, HD, 2, dtype=np.float64) / HD)))
    ang = position_ids.astype(np.float64).reshape(-1)[None, :] * np.concatenate(
        [inv_freq, inv_freq]
    )[:, None]                                           # [hd, tok]
    cosT = np.cos(ang).astype(np.float32)
    sinT = np.sin(ang).astype(np.float32)
    nsinT = (-sinT).copy()
    # mask over fresh keys: maskT[j, (h, t)] = 1 if j <= t (bottom-right causal)
    jj = np.arange(S)[:, None]
    tt = np.tile(np.arange(S)[None, :], (1, HQ)).reshape(1, ROWS)
    maskT = (jj <= tt).astype(np.float32)

    in_maps = []
    for c in range(NCORES):
        q0 = c * HQ * HD
        in_maps.append({
            "hiddenT": hiddenT,
            "wq": np.ascontiguousarray(
                (Wq[:, q0:q0 + HQ * HD] * SCALE).astype(np.float32)
            ).astype(ml_dtypes.bfloat16),
            "wkv": np.ascontiguousarray(
                np.concatenate(
                    [Wk[:, c * HD:(c + 1) * HD], Wv[:, c * HD:(c + 1) * HD]], axis=1
                ), dtype=np.float32).astype(ml_dtypes.bfloat16),
            "wo": np.ascontiguousarray(
                Wo[q0:q0 + HQ * HD, :].astype(np.float32)
            ).astype(ml_dtypes.bfloat16),
            "kT": np.ascontiguousarray(
                k_cache[:, :, c, :].transpose(0, 2, 1)).astype(ml_dtypes.bfloat16),
            # pre-permuted to the sbuf tile layout: v_r[b, p, tt*HD+d] =
            # v[b, tt*128+p, d] -> fully contiguous 8KB DMA rows
            "v": np.ascontiguousarray(
                v_cache[:, :, c, :].reshape(B, PAST // 128, 128, HD)
                .transpose(0, 2, 1, 3).reshape(B, 128, PAST)
            ).astype(ml_dtypes.bfloat16),
            "cosT": cosT, "sinT": sinT, "nsinT": nsinT, "maskT": maskT,
        })
    return in_maps


def kernel(hidden_states, k_cache, v_cache, Wq, Wk, Wv, Wo, position_ids):
    hidden_states = np.asarray(hidden_states)
    nc = _get_nc()
    in_maps = make_in_maps(
        np.asarray(hidden_states), np.asarray(k_cache), np.asarray(v_cache),
        np.asarray(Wq), np.asarray(Wk), np.asarray(Wv), np.asarray(Wo),
        np.asarray(position_ids),
    )
    res = run_bass_kernel_spmd(nc, in_maps, list(range(NCORES)))
    out = np.zeros((TOK, H), np.float32)
    for c in range(NCORES):
        out += res.results[c]["out_p"]
    return out.reshape(B, S, H)



# revision 7
# speedup vs baseline: 1.3624x; 1.3624x over previous
"""Tensor-parallel Llama attention (decode, GQA, RoPE, KV-cache) on 8 TRN2 cores.

Sharding: core c owns kv-head c and q-heads 4c..4c+3. Wq/Wk/Wv are sharded
column-wise, Wo row-wise; each core computes a partial o_proj output and the
host sums the 8 partials (the all-reduce).

Kernel structure (DMA-count minimized; the cost model serializes every DMA
instruction on one HWDGE device for ~625ns, so few-and-huge transfers win):
  - All weights/tables are host-prepacked into the exact SBUF layout so each
    is ONE dma with maximal contiguous descriptors: consts, hT, wkv, wq, wo,
    and per-batch kT/v (15 loads + 1 store total vs 129 before).
  - Issue order on the sync queue keeps DMA_ENGINES busy back-to-back:
    consts, hT, wkv, wq, kT0, v0, kT1, v1, wo, kT2, v2, kT3, v3a, v3b.
  - qT is produced directly by the projection (stationary wq chunk, moving
    hidden chunk) - no PE transposes.
  - Softmax without max-subtraction (|score| <= ~8, fp32 exp safe): exp tiles
    accumulate via DVE, a ones-column matmul reduces the denominator.
  - o_proj runs per batch as soon as that batch is normalized (tokens of
    batch b only need batch b's attention), so only batch 3's slice is on the
    tail. Output is produced transposed ([m, (mc,b,t)]) for full-width PE.
"""

import numpy as np
import ml_dtypes

import concourse.bass as bass
import concourse.mybir as mybir
import concourse.tile as tile
from concourse import bacc
from concourse.bass_utils import run_bass_kernel_spmd

F32 = mybir.dt.float32
BF16 = mybir.dt.bfloat16
AF = mybir.ActivationFunctionType

# Problem shape (hardcoded per contract)
B, S, H = 4, 16, 4096
NH, NKV, HD = 32, 8, 128
PAST = 8192
ROPE_BASE = 10000.0
NCORES = 8
HQ = NH // NCORES          # q heads per core = 4
TOK = B * S                # 64 tokens
NCH = H // 128             # 32 contraction chunks for projections
ROWS = HQ * S              # 64 (head, token) query rows per batch
SCALE = HD ** -0.5
KTILES = PAST // 128       # 64 kpos tiles per batch
GRP = 8                    # kpos tiles per score/exp group ([128, 512] psum)
MC = H // 128              # 32 output column chunks for o_proj


def build_nc(b=B, s=S, h=H, hq=HQ, hd=HD, past=PAST):
    tok = b * s
    nch = h // 128
    rows = hq * s
    ktiles = past // 128

    nc = bacc.Bacc("TRN2", target_bir_lowering=False, debug=False)

    # host-prepacked inputs (see make_in_maps for layouts)
    consts_d = nc.dram_tensor("consts", [128, 3 * hq * tok + tok], F32,
                              kind="ExternalInput").ap()
    hT_d = nc.dram_tensor("hT", [128, nch * tok], BF16, kind="ExternalInput").ap()
    wq_d = nc.dram_tensor("wq", [128, nch * hq * hd], BF16, kind="ExternalInput").ap()
    wkv_d = nc.dram_tensor("wkv", [128, nch * 2 * hd], BF16, kind="ExternalInput").ap()
    wo_d = nc.dram_tensor("wo", [128, hq * h], BF16, kind="ExternalInput").ap()
    kT_d = nc.dram_tensor("kT", [b, hd, past], BF16, kind="ExternalInput").ap()
    v_d = nc.dram_tensor("v", [b, 128, past], BF16, kind="ExternalInput").ap()
    out_d = nc.dram_tensor("out_p", [128, (h // 128) * tok], BF16,
                           kind="ExternalOutput").ap()

    half = hd // 2
    jt = hq * tok                    # 256: (j, b, t) col count

    with tile.TileContext(nc) as tc:
        import contextlib

        with contextlib.ExitStack() as ctx:
            ep = ctx.enter_context
            const_p = ep(tc.tile_pool(name="const", bufs=1))
            kv_p = ep(tc.tile_pool(name="kv", bufs=2))
            work_p = ep(tc.tile_pool(name="work", bufs=1))
            exp_p = ep(tc.tile_pool(name="exp", bufs=6))
            acc_p = ep(tc.tile_pool(name="acc", bufs=2))
            # PSUM tags (8 banks): sc(2) qT/score-groups, kv(2) proj-kv +
            # den/bc/scn/vshift, attn(2) per-batch attn acc, oT(2) o_proj
            ps = ep(tc.tile_pool(name="ps", bufs=2, space="PSUM"))

            # ---- DMA issue order (one sync queue; program order == priority)
            consts = const_p.tile([128, 3 * jt + tok], F32)
            nc.sync.dma_start(consts[:], consts_d[:])
            cos4 = consts[:, 0:jt]
            sin4 = consts[:, jt:2 * jt]
            nsin4 = consts[:, 2 * jt:3 * jt]
            maskT = consts[0:s, 3 * jt:3 * jt + tok]

            hT = const_p.tile([128, nch * tok], BF16)
            nc.sync.dma_start(hT[:], hT_d[:])
            wkv_sb = const_p.tile([128, nch * 2 * hd], BF16)
            nc.sync.dma_start(wkv_sb[:], wkv_d[:])
            wq_sb = const_p.tile([128, nch * hq * hd], BF16)
            nc.sync.dma_start(wq_sb[:], wq_d[:])

            kts = []
            vts = []
            wo_sb = None
            for bb in range(b):
                kt = kv_p.tile([128, past], BF16, tag="kt", bufs=2)
                nc.sync.dma_start(kt[:], kT_d[bb])
                kts.append(kt)
                vt = kv_p.tile([128, past], BF16, tag="vt", bufs=3)
                if bb < b - 1:
                    nc.sync.dma_start(vt[:], v_d[bb])
                else:
                    # split the last v so only 1/4 of its attn-v is on the tail
                    cut = (3 * past) // 4
                    nc.sync.dma_start(vt[:, 0:cut], v_d[bb, :, 0:cut])
                    nc.sync.dma_start(vt[:, cut:past], v_d[bb, :, cut:past])
                vts.append(vt)
                if bb == 1:
                    # wo arrives mid-stream: needed first at batch-0 o_proj
                    wo_sb = const_p.tile([128, hq * h], BF16)
                    nc.sync.dma_start(wo_sb[:], wo_d[:])

            # ---- constants ----
            ones_col = const_p.tile([128, 1], F32)
            nc.vector.memset(ones_col[:], 1.0)
            ones_row = const_p.tile([1, 128], F32)
            nc.vector.memset(ones_row[:], 1.0)
            # row-selector identities for the fresh-v partition shift:
            # isel[bb][p, t] = 1 if p == bb*s + t
            isel = const_p.tile([tok, s * b], F32)
            nc.vector.memset(isel[:], 0.0)
            from concourse.masks import make_identity
            for bb in range(b):
                make_identity(
                    nc, isel[bb * s:(bb + 1) * s, bb * s:(bb + 1) * s]
                )

            # ---- projections ----
            # qT_ps[d, (j,b,t)]; kT_ps[d, (b,t)]; v_ps[(b,t), d]
            qT_ps = ps.tile([128, jt], F32, tag="sc")
            kT_ps = ps.tile([128, tok], F32, tag="kv")
            v_ps = ps.tile([tok, hd], F32, tag="kv")
            for c in range(nch):
                h_c = hT[:, c * tok:(c + 1) * tok]
                fl = dict(start=(c == 0), stop=(c == nch - 1))
                for j in range(hq):
                    nc.tensor.matmul(
                        qT_ps[:, j * tok:(j + 1) * tok],
                        wq_sb[:, c * hq * hd + j * hd:c * hq * hd + (j + 1) * hd],
                        h_c, skip_group_check=True, **fl,
                    )
                nc.tensor.matmul(
                    kT_ps[:], wkv_sb[:, c * 2 * hd:c * 2 * hd + hd], h_c,
                    skip_group_check=True, **fl,
                )
                nc.tensor.matmul(
                    v_ps[:], h_c, wkv_sb[:, c * 2 * hd + hd:(c + 1) * 2 * hd],
                    skip_group_check=True, **fl,
                )

            # ---- RoPE ----
            def rope_parts(src, cosv, sinv, nsinv, n):
                t1f = work_p.tile([128, jt], F32, tag="r1", name="r1")
                t1 = t1f[:, 0:n]
                nc.vector.tensor_mul(t1, src, cosv)
                t2f = work_p.tile([128, jt], F32, tag="r2", name="r2")
                t2 = t2f[:, 0:n]
                nc.vector.tensor_mul(t2[0:half, :], src[half:hd, :], nsinv[0:half, :])
                nc.vector.tensor_mul(t2[half:hd, :], src[0:half, :], sinv[half:hd, :])
                return t1, t2

            # qT_sb layout: [d, (b, j, t)] so each batch slice is contiguous;
            # the rope add scatters from the projection's (j, b, t) order.
            qT_sb = work_p.tile([128, jt], F32, tag="qT")
            t1, t2 = rope_parts(qT_ps[:], cos4, sin4, nsin4, jt)
            qdst = qT_sb[:].rearrange("p (bb j t) -> p j bb t", bb=b, j=hq)
            nc.vector.tensor_add(
                qdst,
                t1.rearrange("p (j bb t) -> p j bb t", j=hq, bb=b),
                t2.rearrange("p (j bb t) -> p j bb t", j=hq, bb=b),
            )
            qT_bf = work_p.tile([128, jt], BF16, tag="qTbf")
            nc.vector.tensor_copy(qT_bf[:], qT_sb[:])

            kT_new = work_p.tile([128, tok], F32, tag="kTn")
            t1k, t2k = rope_parts(kT_ps[:], cos4[:, 0:tok], sin4[:, 0:tok],
                                  nsin4[:, 0:tok], tok)
            nc.vector.tensor_add(kT_new[:], t1k, t2k)

            # fresh v: copy out of psum, then PE row-shift each batch slice to
            # partition base 0 (stationary operand base must be 0/32/64/96)
            v_sb = work_p.tile([tok, hd], F32, tag="vsb")
            nc.scalar.copy(v_sb[:], v_ps[:])
            v_new = []
            for bb in range(b):
                sh_ps = ps.tile([s, hd], F32, tag="kv", name=f"vsh{bb}")
                nc.tensor.matmul(
                    sh_ps[:], isel[:, bb * s:(bb + 1) * s], v_sb[:],
                    start=True, stop=True,
                )
                vn = work_p.tile([s, hd], F32, tag=f"vn{bb}", name=f"vn{bb}")
                nc.scalar.copy(vn[:], sh_ps[:])
                v_new.append(vn)

            # ---- attention + per-batch o_proj ----
            attnT_sb = work_p.tile([128, jt], BF16, tag="attnT")  # (j, b, t)
            outT_sb = work_p.tile([128, MC * tok], BF16, tag="outT")  # (mc,b,t)
            for bb in range(b):
                qT_b = qT_bf[:, bb * rows:(bb + 1) * rows]
                qT_b32 = qT_sb[:, bb * rows:(bb + 1) * rows]
                kt = kts[bb]
                attn_ps = ps.tile([128, rows], F32, tag="attn")
                acc = acc_p.tile([128, rows], F32, tag="acc")
                for g in range(ktiles // GRP):
                    sc_ps = ps.tile([128, GRP * rows], F32, tag="sc")
                    for u in range(GRP):
                        tt = g * GRP + u
                        nc.tensor.matmul(
                            sc_ps[:, u * rows:(u + 1) * rows],
                            kt[:, tt * 128:(tt + 1) * 128], qT_b,
                            start=(u == 0), stop=(u == GRP - 1),
                        )
                    ex = exp_p.tile([128, GRP * rows], BF16, tag="ex")
                    nc.scalar.activation(ex[:], sc_ps[:], AF.Exp)
                    red = acc if g == 0 else acc_p.tile(
                        [128, rows], F32, tag="red", name="red")
                    nc.vector.tensor_reduce(
                        red[:],
                        ex[:].rearrange("p (u q) -> p q u", u=GRP),
                        axis=mybir.AxisListType.X, op=mybir.AluOpType.add,
                    )
                    if red is not acc:
                        nc.vector.tensor_add(acc[:], acc[:], red[:])
                    vt = vts[bb]
                    for u in range(GRP):
                        tt = g * GRP + u
                        nc.tensor.matmul(
                            attn_ps[:], vt[:, tt * hd:(tt + 1) * hd],
                            ex[:, u * rows:(u + 1) * rows],
                            start=(tt == 0), stop=False, skip_group_check=True,
                        )
                # fresh keys (the only masked block)
                scn_ps = ps.tile([s, rows], F32, tag="kv", name="scn")
                nc.tensor.matmul(
                    scn_ps[:], kT_new[:, bb * s:(bb + 1) * s], qT_b32,
                    start=True, stop=True,
                )
                exn = exp_p.tile([s, rows], F32, tag="exn")
                nc.scalar.activation(exn[:], scn_ps[:], AF.Exp)
                nc.vector.tensor_mul(exn[:], exn[:], maskT)
                nc.vector.tensor_add(acc[0:s, :], acc[0:s, :], exn[:])
                nc.tensor.matmul(
                    attn_ps[:], v_new[bb][:], exn[:],
                    start=False, stop=True, skip_group_check=True,
                )
                # denominator: reduce acc over partitions, broadcast reciprocal
                dsum_ps = ps.tile([1, rows], F32, tag="kv", name="dsum")
                nc.tensor.matmul(dsum_ps[:], ones_col[:], acc[:],
                                 start=True, stop=True)
                rden = acc_p.tile([1, rows], F32, tag="rden")
                nc.vector.reciprocal(rden[:], dsum_ps[:])
                bc_ps = ps.tile([128, rows], F32, tag="kv", name="bc")
                nc.tensor.matmul(bc_ps[:], ones_row[:], rden[:],
                                 start=True, stop=True)
                rdenb = acc_p.tile([128, rows], F32, tag="rdenb")
                nc.scalar.copy(rdenb[:], bc_ps[:])
                # normalize + scatter (j,t) -> (j, b, t)
                adst = attnT_sb[:].rearrange("p (j bb t) -> p j bb t",
                                             j=hq, bb=b)[:, :, bb, :]
                nc.vector.tensor_mul(
                    adst,
                    attn_ps[:].rearrange("p (j t) -> p j t", j=hq),
                    rdenb[:].rearrange("p (j t) -> p j t", j=hq),
                )
                # o_proj for this batch's 16 token columns:
                # oTb[m, (mc,t)] = sum_j wo_j[:,mc]^T @ attnT_j[:, (bb,t)]
                oTb = ps.tile([128, MC * s], F32, tag="oT")
                for mc in range(MC):
                    for j in range(hq):
                        nc.tensor.matmul(
                            oTb[:, mc * s:(mc + 1) * s],
                            wo_sb[:, j * h + mc * 128:j * h + (mc + 1) * 128],
                            attnT_sb[:, j * tok + bb * s:j * tok + (bb + 1) * s],
                            start=(j == 0), stop=(j == hq - 1),
                            skip_group_check=True,
                        )
                odst = outT_sb[:].rearrange("p (mc bb t) -> p bb mc t",
                                            mc=MC, bb=b)[:, bb, :, :]
                nc.vector.tensor_copy(
                    odst, oTb[:].rearrange("p (mc t) -> p mc t", mc=MC),
                )

            nc.sync.dma_start(out_d[:], outT_sb[:])

    nc.compile()
    return nc


_NC_CACHE = {}


def _get_nc(key=(B, S, H, HQ, HD, PAST)):
    if key not in _NC_CACHE:
        _NC_CACHE[key] = build_nc(*key)
    return _NC_CACHE[key]


def make_in_maps(hidden_states, k_cache, v_cache, Wq, Wk, Wv, Wo, position_ids):
    """Host-side shard + layout prep: one input dict per core."""
    bf = ml_dtypes.bfloat16
    hid = hidden_states.reshape(TOK, H).astype(np.float32)
    # hT[p, c*TOK + t] = hidden[t, c*128+p]
    hT = np.ascontiguousarray(
        hid.T.reshape(NCH, 128, TOK).transpose(1, 0, 2).reshape(128, NCH * TOK)
    ).astype(bf)
    # RoPE tables, tiled 4x over j: [128, (j, b, t)]
    inv_freq = 1.0 / (ROPE_BASE ** (np.arange(0, HD, 2, dtype=np.float64) / HD))
    ang = position_ids.astype(np.float64).reshape(-1)[None, :] * np.concatenate(
        [inv_freq, inv_freq])[:, None]                      # [hd, tok]
    cosT = np.cos(ang).astype(np.float32)
    sinT = np.sin(ang).astype(np.float32)
    cos4 = np.tile(cosT, (1, HQ))
    sin4 = np.tile(sinT, (1, HQ))
    nsin4 = -sin4
    # mask over fresh keys: maskT[j, (h, t)] = 1 if j <= t
    jj = np.arange(S)[:, None]
    tt = np.tile(np.arange(S)[None, :], (1, HQ)).reshape(1, ROWS)
    maskkT = np.zeros((128, TOK), np.float32)
    maskkT[0:S, :] = (jj <= tt).astype(np.float32)
    consts = np.ascontiguousarray(
        np.concatenate([cos4, sin4, nsin4, maskkT], axis=1))

    in_maps = []
    for c in range(NCORES):
        q0 = c * HQ * HD
        # wq[p, (c, j, d)] = Wq[c*128+p, q0 + j*128 + d] * SCALE
        wq = np.ascontiguousarray(
            (Wq[:, q0:q0 + HQ * HD] * SCALE).astype(np.float32)
            .reshape(NCH, 128, HQ * HD).transpose(1, 0, 2).reshape(128, -1)
        ).astype(bf)
        # wkv[p, (c, {k:0,v:1}, d)]
        wkv = np.concatenate(
            [Wk[:, c * HD:(c + 1) * HD].reshape(NCH, 128, HD),
             Wv[:, c * HD:(c + 1) * HD].reshape(NCH, 128, HD)], axis=2
        ).astype(np.float32).transpose(1, 0, 2).reshape(128, NCH * 2 * HD)
        wkv = np.ascontiguousarray(wkv).astype(bf)
        # wo[p, (j, m)] = Wo[q0 + j*128 + p, m]
        wo = np.ascontiguousarray(
            Wo[q0:q0 + HQ * HD, :].astype(np.float32)
            .reshape(HQ, 128, H).transpose(1, 0, 2).reshape(128, HQ * H)
        ).astype(bf)
        in_maps.append({
            "consts": consts,
            "hT": hT,
            "wq": wq,
            "wkv": wkv,
            "wo": wo,
            "kT": np.ascontiguousarray(
                k_cache[:, :, c, :].transpose(0, 2, 1)).astype(bf),
            # v_r[b, p, tt*HD+d] = v[b, tt*128+p, d]
            "v": np.ascontiguousarray(
                v_cache[:, :, c, :].reshape(B, PAST // 128, 128, HD)
                .transpose(0, 2, 1, 3).reshape(B, 128, PAST)
            ).astype(bf),
        })
    return in_maps


def kernel(hidden_states, k_cache, v_cache, Wq, Wk, Wv, Wo, position_ids):
    nc = _get_nc()
    in_maps = make_in_maps(
        np.asarray(hidden_states), np.asarray(k_cache), np.asarray(v_cache),
        np.asarray(Wq), np.asarray(Wk), np.asarray(Wv), np.asarray(Wo),
        np.asarray(position_ids),
    )
    res = run_bass_kernel_spmd(nc, in_maps, list(range(NCORES)))
    # out_p[p, (mc, b, t)] -> out[(b,t), mc*128+p]; host sums the 8 partials
    acc = np.zeros((128, MC * TOK), np.float32)
    for c in range(NCORES):
        acc += res.results[c]["out_p"].astype(np.float32)
    out = acc.reshape(128, MC, TOK).transpose(2, 1, 0).reshape(TOK, H)
    return np.ascontiguousarray(out).reshape(B, S, H)


# revision 12
# speedup vs baseline: 1.3731x; 1.0079x over previous
"""Tensor-parallel Llama attention (decode, GQA, RoPE, KV-cache) on 8 TRN2 cores.

Sharding: core c owns kv-head c and q-heads 4c..4c+3. Wq/Wk/Wv are sharded
column-wise, Wo row-wise; each core computes a partial o_proj output and the
host sums the 8 partials (the all-reduce).

Kernel structure (DMA-count minimized; the cost model serializes every DMA
instruction on one HWDGE device for ~625ns, so few-and-huge transfers win):
  - All weights/tables are host-prepacked into the exact SBUF layout so each
    is ONE dma with maximal contiguous descriptors: consts, hT, wkv, wq, wo,
    and per-batch kT/v (15 loads + 1 store total vs 129 before).
  - Issue order on the sync queue keeps DMA_ENGINES busy back-to-back:
    consts, hT, wkv, wq, kT0, v0, kT1, v1, wo, kT2, v2, kT3, v3a, v3b.
  - qT is produced directly by the projection (stationary wq chunk, moving
    hidden chunk) - no PE transposes.
  - Softmax without max-subtraction (|score| <= ~8, fp32 exp safe): exp tiles
    accumulate via DVE, a ones-column matmul reduces the denominator.
  - o_proj runs per batch as soon as that batch is normalized (tokens of
    batch b only need batch b's attention), so only batch 3's slice is on the
    tail. Output is produced transposed ([m, (mc,b,t)]) for full-width PE.
"""

import numpy as np
import ml_dtypes

import concourse.bass as bass
import concourse.mybir as mybir
import concourse.tile as tile
from concourse import bacc
from concourse.bass_utils import run_bass_kernel_spmd

F32 = mybir.dt.float32
BF16 = mybir.dt.bfloat16
AF = mybir.ActivationFunctionType

# Problem shape (hardcoded per contract)
B, S, H = 4, 16, 4096
NH, NKV, HD = 32, 8, 128
PAST = 8192
ROPE_BASE = 10000.0
NCORES = 8
HQ = NH // NCORES          # q heads per core = 4
TOK = B * S                # 64 tokens
NCH = H // 128             # 32 contraction chunks for projections
ROWS = HQ * S              # 64 (head, token) query rows per batch
SCALE = HD ** -0.5
KTILES = PAST // 128       # 64 kpos tiles per batch
GRP = 8                    # kpos tiles per score/exp group ([128, 512] psum)
MC = H // 128              # 32 output column chunks for o_proj


def build_nc(b=B, s=S, h=H, hq=HQ, hd=HD, past=PAST):
    tok = b * s
    nch = h // 128
    rows = hq * s
    ktiles = past // 128

    nc = bacc.Bacc("TRN2", target_bir_lowering=False, debug=False)

    # host-prepacked inputs (see make_in_maps for layouts)
    consts_d = nc.dram_tensor("consts", [128, 3 * hq * tok + tok], F32,
                              kind="ExternalInput").ap()
    hT_d = nc.dram_tensor("hT", [128, nch * tok], BF16, kind="ExternalInput").ap()
    wq_d = nc.dram_tensor("wq", [128, nch * hq * hd], BF16, kind="ExternalInput").ap()
    wkv_d = nc.dram_tensor("wkv", [128, nch * 2 * hd], BF16, kind="ExternalInput").ap()
    wo_d = nc.dram_tensor("wo", [128, hq * h], BF16, kind="ExternalInput").ap()
    kT_d = nc.dram_tensor("kT", [b, hd, past], BF16, kind="ExternalInput").ap()
    v_d = nc.dram_tensor("v", [b, 128, past], BF16, kind="ExternalInput").ap()
    out_d = nc.dram_tensor("out_p", [128, (h // 128) * tok], BF16,
                           kind="ExternalOutput").ap()

    half = hd // 2
    jt = hq * tok                    # 256: (j, b, t) col count

    with tile.TileContext(nc) as tc:
        import contextlib

        with contextlib.ExitStack() as ctx:
            ep = ctx.enter_context
            const_p = ep(tc.tile_pool(name="const", bufs=1))
            kv_p = ep(tc.tile_pool(name="kv", bufs=2))
            work_p = ep(tc.tile_pool(name="work", bufs=1))
            exp_p = ep(tc.tile_pool(name="exp", bufs=6))
            acc_p = ep(tc.tile_pool(name="acc", bufs=2))
            # PSUM tags (8 banks): sc(2) qT/score-groups, kv(2) proj-kv +
            # den/bc/scn/vshift, attn(2) per-batch attn acc, oT(2) o_proj
            ps = ep(tc.tile_pool(name="ps", bufs=2, space="PSUM"))

            # ---- DMA issue order (one sync queue; program order == priority)
            consts = const_p.tile([128, 3 * jt + tok], F32)
            nc.sync.dma_start(consts[:], consts_d[:])
            cos4 = consts[:, 0:jt]
            sin4 = consts[:, jt:2 * jt]
            nsin4 = consts[:, 2 * jt:3 * jt]
            maskT = consts[0:s, 3 * jt:3 * jt + tok]

            hT = const_p.tile([128, nch * tok], BF16)
            nc.sync.dma_start(hT[:], hT_d[:])
            wkv_sb = const_p.tile([128, nch * 2 * hd], BF16)
            nc.sync.dma_start(wkv_sb[:], wkv_d[:])
            wq_sb = const_p.tile([128, nch * hq * hd], BF16)
            nc.sync.dma_start(wq_sb[:], wq_d[:])

            # kv stream order: kT0 v0 kT1 v1 wo kT2 kT3 v2 v3a v3b.
            # kT3 ahead of v2 lets batch 3's score/exp chain (the longest
            # serial Act chain) run during the v2/v3 transfers; the split v3
            # tail chunk leaves only 16 attn-v tiles + o_proj on the tail.
            kts = []
            vts = []
            for bb in range(b):
                kt = kv_p.tile([128, past], BF16, tag="kt", bufs=2, name=f"kt{bb}")
                nc.sync.dma_start(kt[:], kT_d[bb])
                kts.append(kt)
                vt = kv_p.tile([128, past], BF16, tag="vt", bufs=3, name=f"vt{bb}")
                vts.append(vt)
                if bb < 2:
                    nc.sync.dma_start(vt[:], v_d[bb])
                if bb == 1:
                    # wo arrives mid-stream: needed first at batch-0 o_proj
                    wo_sb = const_p.tile([128, hq * h], BF16)
                    nc.sync.dma_start(wo_sb[:], wo_d[:])
            nc.sync.dma_start(vts[2][:], v_d[2])
            cut = (3 * past) // 4
            nc.sync.dma_start(vts[3][:, 0:cut], v_d[3, :, 0:cut])
            nc.sync.dma_start(vts[3][:, cut:past], v_d[3, :, cut:past])

            # ---- constants ----
            ones_col = const_p.tile([128, 1], F32)
            nc.vector.memset(ones_col[:], 1.0)
            ones_bf = const_p.tile([128, 1], BF16)
            nc.vector.memset(ones_bf[:], 1.0)
            ones_row = const_p.tile([1, 128], F32)
            nc.vector.memset(ones_row[:], 1.0)
            # row-selector identities for the fresh-v partition shift:
            # isel[bb][p, t] = 1 if p == bb*s + t
            isel = const_p.tile([tok, s * b], F32)
            nc.vector.memset(isel[:], 0.0)
            from concourse.masks import make_identity
            for bb in range(b):
                make_identity(
                    nc, isel[bb * s:(bb + 1) * s, bb * s:(bb + 1) * s]
                )

            # ---- projections ----
            # qT_ps[d, (j,b,t)]; kT_ps[d, (b,t)]; v_ps[(b,t), d]
            qT_ps = ps.tile([128, jt], F32, tag="sc")
            kT_ps = ps.tile([128, tok], F32, tag="kv")
            v_ps = ps.tile([tok, hd], F32, tag="kv")
            for c in range(nch):
                h_c = hT[:, c * tok:(c + 1) * tok]
                fl = dict(start=(c == 0), stop=(c == nch - 1))
                for j in range(hq):
                    nc.tensor.matmul(
                        qT_ps[:, j * tok:(j + 1) * tok],
                        wq_sb[:, c * hq * hd + j * hd:c * hq * hd + (j + 1) * hd],
                        h_c, skip_group_check=True, **fl,
                    )
                nc.tensor.matmul(
                    kT_ps[:], wkv_sb[:, c * 2 * hd:c * 2 * hd + hd], h_c,
                    skip_group_check=True, **fl,
                )
                nc.tensor.matmul(
                    v_ps[:], h_c, wkv_sb[:, c * 2 * hd + hd:(c + 1) * 2 * hd],
                    skip_group_check=True, **fl,
                )

            # ---- RoPE ----
            def rope_parts(src, cosv, sinv, nsinv, n):
                t1f = work_p.tile([128, jt], F32, tag="r1", name="r1")
                t1 = t1f[:, 0:n]
                nc.vector.tensor_mul(t1, src, cosv)
                t2f = work_p.tile([128, jt], F32, tag="r2", name="r2")
                t2 = t2f[:, 0:n]
                nc.vector.tensor_mul(t2[0:half, :], src[half:hd, :], nsinv[0:half, :])
                nc.vector.tensor_mul(t2[half:hd, :], src[0:half, :], sinv[half:hd, :])
                return t1, t2

            # qT_sb layout: [d, (b, j, t)] so each batch slice is contiguous;
            # the rope add scatters from the projection's (j, b, t) order.
            qT_sb = work_p.tile([128, jt], F32, tag="qT")
            t1, t2 = rope_parts(qT_ps[:], cos4, sin4, nsin4, jt)
            qdst = qT_sb[:].rearrange("p (bb j t) -> p j bb t", bb=b, j=hq)
            nc.vector.tensor_add(
                qdst,
                t1.rearrange("p (j bb t) -> p j bb t", j=hq, bb=b),
                t2.rearrange("p (j bb t) -> p j bb t", j=hq, bb=b),
            )
            qT_bf = work_p.tile([128, jt], BF16, tag="qTbf")
            nc.vector.tensor_copy(qT_bf[:], qT_sb[:])

            kT_new = work_p.tile([128, tok], F32, tag="kTn")
            t1k, t2k = rope_parts(kT_ps[:], cos4[:, 0:tok], sin4[:, 0:tok],
                                  nsin4[:, 0:tok], tok)
            nc.vector.tensor_add(kT_new[:], t1k, t2k)

            # fresh v: copy out of psum, then PE row-shift each batch slice to
            # partition base 0 (stationary operand base must be 0/32/64/96)
            v_sb = work_p.tile([tok, hd], F32, tag="vsb")
            nc.scalar.copy(v_sb[:], v_ps[:])
            v_new = []
            for bb in range(b):
                sh_ps = ps.tile([s, hd], F32, tag="kv", name=f"vsh{bb}")
                nc.tensor.matmul(
                    sh_ps[:], isel[:, bb * s:(bb + 1) * s], v_sb[:],
                    start=True, stop=True,
                )
                vn = work_p.tile([s, hd], F32, tag=f"vn{bb}", name=f"vn{bb}")
                nc.scalar.copy(vn[:], sh_ps[:])
                v_new.append(vn)

            # ---- attention + per-batch o_proj ----
            # Denominator via PE: den_ps[1, (j,t)] accumulates column sums of
            # every exp tile (+ masked fresh exp). It needs no v, so for each
            # batch it completes right after the score/exp chain: recip and
            # the broadcast run during the kv transfers, leaving only the
            # last attn-v chunk, normalize and o_proj after the last v byte.
            # The den matmuls are issued AFTER the whole score/exp chain so
            # the in-order PE never waits mid-chain on the Act engine.
            attnT_sb = work_p.tile([128, jt], BF16, tag="attnT")  # (j, b, t)
            outT_sb = work_p.tile([128, MC * tok], BF16, tag="outT")  # (mc,b,t)
            for bb in range(b):
                qT_b = qT_bf[:, bb * rows:(bb + 1) * rows]
                qT_b32 = qT_sb[:, bb * rows:(bb + 1) * rows]
                kt = kts[bb]
                vt = vts[bb]
                exs = []
                for g in range(ktiles // GRP):
                    sc_ps = ps.tile([128, GRP * rows], F32, tag="sc",
                                    name=f"sc{bb}_{g}")
                    for u in range(GRP):
                        tt = g * GRP + u
                        nc.tensor.matmul(
                            sc_ps[:, u * rows:(u + 1) * rows],
                            kt[:, tt * 128:(tt + 1) * 128], qT_b,
                            start=(u == 0), stop=(u == GRP - 1),
                        )
                    ex = exp_p.tile([128, GRP * rows], BF16, tag="ex",
                                    name=f"ex{bb}_{g}", bufs=12)
                    nc.scalar.activation(ex[:], sc_ps[:], AF.Exp)
                    exs.append(ex)
                # fresh keys (the only masked block)
                scn_ps = ps.tile([s, rows], F32, tag="kv", name="scn")
                nc.tensor.matmul(
                    scn_ps[:], kT_new[:, bb * s:(bb + 1) * s], qT_b32,
                    start=True, stop=True,
                )
                exn = exp_p.tile([s, rows], F32, tag="exn")
                nc.scalar.activation(exn[:], scn_ps[:], AF.Exp)
                nc.vector.tensor_mul(exn[:], exn[:], maskT)
                # denominator accumulation (PE, reads only exp tiles)
                den_ps = ps.tile([1, rows], F32, tag="kv", name="den")
                for g in range(ktiles // GRP):
                    for u in range(GRP):
                        nc.tensor.matmul(
                            den_ps[:], ones_bf[:],
                            exs[g][:, u * rows:(u + 1) * rows],
                            start=(g == 0 and u == 0), stop=False,
                            skip_group_check=True,
                        )
                nc.tensor.matmul(
                    den_ps[:], ones_col[0:s, :], exn[:],
                    start=False, stop=True, skip_group_check=True,
                )
                # reciprocal + broadcast over partitions (all pre-v work)
                rden = acc_p.tile([1, rows], F32, tag="rden")
                nc.vector.reciprocal(rden[:], den_ps[:])
                bc_ps = ps.tile([128, rows], F32, tag="kv", name="bc")
                nc.tensor.matmul(bc_ps[:], ones_row[:], rden[:],
                                 start=True, stop=True)
                rdenb = acc_p.tile([128, rows], F32, tag="rdenb")
                nc.scalar.copy(rdenb[:], bc_ps[:])

                # attn-v in one chunk (batches 0-2) or two (batch 3): the
                # first chunk's normalize+o_proj overlap the final v transfer
                oTb = ps.tile([128, MC * s], F32, tag="oT")
                chunks = [(0, ktiles)] if bb < b - 1 else [
                    (0, (3 * ktiles) // 4), ((3 * ktiles) // 4, ktiles)]
                for (ct0, ct1) in chunks:
                    last = ct1 == ktiles
                    attn_ps = ps.tile([128, rows], F32, tag="attn",
                                      name=f"attn{bb}_{ct0}")
                    for tt in range(ct0, ct1):
                        nc.tensor.matmul(
                            attn_ps[:], vt[:, tt * hd:(tt + 1) * hd],
                            exs[tt // GRP][:, (tt % GRP) * rows:
                                           (tt % GRP + 1) * rows],
                            start=(tt == ct0),
                            stop=(not last and tt == ct1 - 1),
                            skip_group_check=True,
                        )
                    if last:
                        nc.tensor.matmul(
                            attn_ps[:], v_new[bb][:], exn[:],
                            start=False, stop=True, skip_group_check=True,
                        )
                    # normalize; chunk sums are disjoint so each chunk gets
                    # its own normalized tile feeding its own o_proj pass
                    if ct0 == 0:
                        adst = attnT_sb[:].rearrange(
                            "p (j bb t) -> p j bb t", j=hq, bb=b)[:, :, bb, :]
                        nc.vector.tensor_mul(
                            adst,
                            attn_ps[:].rearrange("p (j t) -> p j t", j=hq),
                            rdenb[:].rearrange("p (j t) -> p j t", j=hq),
                        )
                    else:
                        attnTB = work_p.tile([128, rows], BF16, tag="attnTB")
                        nc.vector.tensor_mul(attnTB[:], attn_ps[:], rdenb[:])
                    # o_proj pass for this chunk (accumulates across chunks)
                    for mc in range(MC):
                        for j in range(hq):
                            if ct0 == 0:
                                rhs = attnT_sb[:, j * tok + bb * s:
                                               j * tok + (bb + 1) * s]
                            else:
                                rhs = attnTB[:, j * s:(j + 1) * s]
                            nc.tensor.matmul(
                                oTb[:, mc * s:(mc + 1) * s],
                                wo_sb[:, j * h + mc * 128:
                                      j * h + (mc + 1) * 128],
                                rhs,
                                start=(j == 0 and ct0 == 0),
                                stop=(j == hq - 1 and last),
                                skip_group_check=True,
                            )
                odst = outT_sb[:].rearrange("p (mc bb t) -> p bb mc t",
                                            mc=MC, bb=b)[:, bb, :, :]
                nc.vector.tensor_copy(
                    odst, oTb[:].rearrange("p (mc t) -> p mc t", mc=MC),
                )

            nc.sync.dma_start(out_d[:], outT_sb[:])

    nc.compile()
    return nc


_NC_CACHE = {}


def _get_nc(key=(B, S, H, HQ, HD, PAST)):
    if key not in _NC_CACHE:
        _NC_CACHE[key] = build_nc(*key)
    return _NC_CACHE[key]


def make_in_maps(hidden_states, k_cache, v_cache, Wq, Wk, Wv, Wo, position_ids):
    """Host-side shard + layout prep: one input dict per core."""
    bf = ml_dtypes.bfloat16
    hid = hidden_states.reshape(TOK, H).astype(np.float32)
    # hT[p, c*TOK + t] = hidden[t, c*128+p]
    hT = np.ascontiguousarray(
        hid.T.reshape(NCH, 128, TOK).transpose(1, 0, 2).reshape(128, NCH * TOK)
    ).astype(bf)
    # RoPE tables, tiled 4x over j: [128, (j, b, t)]
    inv_freq = 1.0 / (ROPE_BASE ** (np.arange(0, HD, 2, dtype=np.float64) / HD))
    ang = position_ids.astype(np.float64).reshape(-1)[None, :] * np.concatenate(
        [inv_freq, inv_freq])[:, None]                      # [hd, tok]
    cosT = np.cos(ang).astype(np.float32)
    sinT = np.sin(ang).astype(np.float32)
    cos4 = np.tile(cosT, (1, HQ))
    sin4 = np.tile(sinT, (1, HQ))
    nsin4 = -sin4
    # mask over fresh keys: maskT[j, (h, t)] = 1 if j <= t
    jj = np.arange(S)[:, None]
    tt = np.tile(np.arange(S)[None, :], (1, HQ)).reshape(1, ROWS)
    maskkT = np.zeros((128, TOK), np.float32)
    maskkT[0:S, :] = (jj <= tt).astype(np.float32)
    consts = np.ascontiguousarray(
        np.concatenate([cos4, sin4, nsin4, maskkT], axis=1))

    in_maps = []
    for c in range(NCORES):
        q0 = c * HQ * HD
        # wq[p, (c, j, d)] = Wq[c*128+p, q0 + j*128 + d] * SCALE
        wq = np.ascontiguousarray(
            (Wq[:, q0:q0 + HQ * HD] * SCALE).astype(np.float32)
            .reshape(NCH, 128, HQ * HD).transpose(1, 0, 2).reshape(128, -1)
        ).astype(bf)
        # wkv[p, (c, {k:0,v:1}, d)]
        wkv = np.concatenate(
            [Wk[:, c * HD:(c + 1) * HD].reshape(NCH, 128, HD),
             Wv[:, c * HD:(c + 1) * HD].reshape(NCH, 128, HD)], axis=2
        ).astype(np.float32).transpose(1, 0, 2).reshape(128, NCH * 2 * HD)
        wkv = np.ascontiguousarray(wkv).astype(bf)
        # wo[p, (j, m)] = Wo[q0 + j*128 + p, m]
        wo = np.ascontiguousarray(
            Wo[q0:q0 + HQ * HD, :].astype(np.float32)
            .reshape(HQ, 128, H).transpose(1, 0, 2).reshape(128, HQ * H)
        ).astype(bf)
        in_maps.append({
            "consts": consts,
            "hT": hT,
            "wq": wq,
            "wkv": wkv,
            "wo": wo,
            "kT": np.ascontiguousarray(
                k_cache[:, :, c, :].transpose(0, 2, 1)).astype(bf),
            # v_r[b, p, tt*HD+d] = v[b, tt*128+p, d]
            "v": np.ascontiguousarray(
                v_cache[:, :, c, :].reshape(B, PAST // 128, 128, HD)
                .transpose(0, 2, 1, 3).reshape(B, 128, PAST)
            ).astype(bf),
        })
    return in_maps


def kernel(hidden_states, k_cache, v_cache, Wq, Wk, Wv, Wo, position_ids):
    nc = _get_nc()
    in_maps = make_in_maps(
        np.asarray(hidden_states), np.asarray(k_cache), np.asarray(v_cache),
        np.asarray(Wq), np.asarray(Wk), np.asarray(Wv), np.asarray(Wo),
        np.asarray(position_ids),
    )
    res = run_bass_kernel_spmd(nc, in_maps, list(range(NCORES)))
    # out_p[p, (mc, b, t)] -> out[(b,t), mc*128+p]; host sums the 8 partials
    acc = np.zeros((128, MC * TOK), np.float32)
    for c in range(NCORES):
        acc += res.results[c]["out_p"].astype(np.float32)
    out = acc.reshape(128, MC, TOK).transpose(2, 1, 0).reshape(TOK, H)
    return np.ascontiguousarray(out).reshape(B, S, H)


# revision 13
# speedup vs baseline: 1.4385x; 1.0476x over previous
"""Tensor-parallel Llama attention (decode, GQA, RoPE, KV-cache) on 8 TRN2 cores.

Sharding: core c owns kv-head c and q-heads 4c..4c+3. Wq/Wk/Wv are sharded
column-wise, Wo row-wise; each core computes a partial o_proj output and the
host sums the 8 partials (the all-reduce).

Kernel structure (DMA-count minimized; the cost model serializes every DMA
instruction on one HWDGE device for ~625ns, so few-and-huge transfers win):
  - All weights/tables are host-prepacked into the exact SBUF layout so each
    is ONE dma with maximal contiguous descriptors: consts, hT, wkv, wq, wo,
    and per-batch kT/v (15 loads + 1 store total vs 129 before).
  - Issue order on the sync queue keeps DMA_ENGINES busy back-to-back:
    consts, hT, wkv, wq, kT0, v0, kT1, v1, wo, kT2, v2, kT3, v3a, v3b.
  - qT is produced directly by the projection (stationary wq chunk, moving
    hidden chunk) - no PE transposes.
  - Softmax without max-subtraction (|score| <= ~8, fp32 exp safe): exp tiles
    accumulate via DVE, a ones-column matmul reduces the denominator.
  - o_proj runs per batch as soon as that batch is normalized (tokens of
    batch b only need batch b's attention), so only batch 3's slice is on the
    tail. Output is produced transposed ([m, (mc,b,t)]) for full-width PE.
"""

import numpy as np
import ml_dtypes

import concourse.bass as bass
import concourse.mybir as mybir
import concourse.tile as tile
from concourse import bacc
from concourse.bass_utils import run_bass_kernel_spmd

F32 = mybir.dt.float32
BF16 = mybir.dt.bfloat16
AF = mybir.ActivationFunctionType

# Problem shape (hardcoded per contract)
B, S, H = 4, 16, 4096
NH, NKV, HD = 32, 8, 128
PAST = 8192
ROPE_BASE = 10000.0
NCORES = 8
HQ = NH // NCORES          # q heads per core = 4
TOK = B * S                # 64 tokens
NCH = H // 128             # 32 contraction chunks for projections
ROWS = HQ * S              # 64 (head, token) query rows per batch
SCALE = HD ** -0.5
KTILES = PAST // 128       # 64 kpos tiles per batch
GRP = 8                    # kpos tiles per score/exp group ([128, 512] psum)
MC = H // 128              # 32 output column chunks for o_proj


def build_nc(b=B, s=S, h=H, hq=HQ, hd=HD, past=PAST):
    tok = b * s
    nch = h // 128
    rows = hq * s
    ktiles = past // 128

    nc = bacc.Bacc("TRN2", target_bir_lowering=False, debug=False)

    # host-prepacked inputs (see make_in_maps for layouts)
    consts_d = nc.dram_tensor("consts", [128, 3 * hq * tok + tok], F32,
                              kind="ExternalInput").ap()
    hT_d = nc.dram_tensor("hT", [128, nch * tok], BF16, kind="ExternalInput").ap()
    wq_d = nc.dram_tensor("wq", [128, nch * hq * hd], BF16, kind="ExternalInput").ap()
    wkv_d = nc.dram_tensor("wkv", [128, nch * 2 * hd], BF16, kind="ExternalInput").ap()
    wo_d = nc.dram_tensor("wo", [128, hq * h], BF16, kind="ExternalInput").ap()
    kT_d = nc.dram_tensor("kT", [b, hd, past], BF16, kind="ExternalInput").ap()
    v_d = nc.dram_tensor("v", [b, 128, past], BF16, kind="ExternalInput").ap()
    out_d = nc.dram_tensor("out_p", [128, (h // 128) * tok], BF16,
                           kind="ExternalOutput").ap()

    half = hd // 2
    jt = hq * tok                    # 256: (j, b, t) col count

    with tile.TileContext(nc) as tc:
        import contextlib

        with contextlib.ExitStack() as ctx:
            ep = ctx.enter_context
            const_p = ep(tc.tile_pool(name="const", bufs=1))
            kv_p = ep(tc.tile_pool(name="kv", bufs=2))
            work_p = ep(tc.tile_pool(name="work", bufs=1))
            exp_p = ep(tc.tile_pool(name="exp", bufs=6))
            acc_p = ep(tc.tile_pool(name="acc", bufs=2))
            # PSUM tags (8 banks): sc(2) qT/score-groups, kv(2) proj-kv +
            # den/bc/scn/vshift, attn(2) per-batch attn acc, oT(2) o_proj
            ps = ep(tc.tile_pool(name="ps", bufs=2, space="PSUM"))

            # ---- DMA issue order (one sync queue; program order == priority)
            consts = const_p.tile([128, 3 * jt + tok], F32)
            nc.sync.dma_start(consts[:], consts_d[:])
            cos4 = consts[:, 0:jt]
            sin4 = consts[:, jt:2 * jt]
            nsin4 = consts[:, 2 * jt:3 * jt]
            maskT = consts[0:s, 3 * jt:3 * jt + tok]

            hT = const_p.tile([128, nch * tok], BF16)
            nc.sync.dma_start(hT[:], hT_d[:])
            wkv_sb = const_p.tile([128, nch * 2 * hd], BF16)
            nc.sync.dma_start(wkv_sb[:], wkv_d[:])
            wq_sb = const_p.tile([128, nch * hq * hd], BF16)
            nc.sync.dma_start(wq_sb[:], wq_d[:])

            # kv stream order: kT0 v0 kT1 v1 kT2 kT3 v2 v3 wo0..wo7.
            # All attention (which chases kT/v arrivals) finishes mid-stream;
            # o_proj output-column chunks then chase the 8 wo chunks, so the
            # tail after the last byte is one tiny o_proj pass + copy + store.
            kts = []
            vts = []
            for bb in range(b):
                kt = kv_p.tile([128, past], BF16, tag="kt", bufs=2, name=f"kt{bb}")
                nc.sync.dma_start(kt[:], kT_d[bb])
                kts.append(kt)
                vt = kv_p.tile([128, past], BF16, tag="vt", bufs=3, name=f"vt{bb}")
                vts.append(vt)
                if bb < 2:
                    nc.sync.dma_start(vt[:], v_d[bb])
            nc.sync.dma_start(vts[2][:], v_d[2])
            nc.sync.dma_start(vts[3][:], v_d[3])
            # wo chunks: mc-counts sum to 32, small chunks last for the tail
            WO_CHUNKS = [5, 5, 5, 5, 4, 4, 2, 2]
            wo_tiles = []
            off = 0
            for g, mcg in enumerate(WO_CHUNKS):
                wog = const_p.tile([128, mcg * 4 * 128], BF16, name=f"wo{g}")
                nc.sync.dma_start(wog[:], wo_d[:, off * 512:(off + mcg) * 512])
                wo_tiles.append((wog, off, mcg))
                off += mcg

            # ---- constants ----
            ones_col = const_p.tile([128, 1], F32)
            nc.vector.memset(ones_col[:], 1.0)
            ones_bf = const_p.tile([128, 1], BF16)
            nc.vector.memset(ones_bf[:], 1.0)
            ones_row = const_p.tile([1, 128], F32)
            nc.vector.memset(ones_row[:], 1.0)
            # row-selector identities for the fresh-v partition shift:
            # isel[bb][p, t] = 1 if p == bb*s + t
            isel = const_p.tile([tok, s * b], F32)
            nc.vector.memset(isel[:], 0.0)
            from concourse.masks import make_identity
            for bb in range(b):
                make_identity(
                    nc, isel[bb * s:(bb + 1) * s, bb * s:(bb + 1) * s]
                )

            # ---- projections ----
            # qT_ps[d, (j,b,t)]; kT_ps[d, (b,t)]; v_ps[(b,t), d]
            qT_ps = ps.tile([128, jt], F32, tag="sc")
            kT_ps = ps.tile([128, tok], F32, tag="kv")
            v_ps = ps.tile([tok, hd], F32, tag="kv")
            for c in range(nch):
                h_c = hT[:, c * tok:(c + 1) * tok]
                fl = dict(start=(c == 0), stop=(c == nch - 1))
                for j in range(hq):
                    nc.tensor.matmul(
                        qT_ps[:, j * tok:(j + 1) * tok],
                        wq_sb[:, c * hq * hd + j * hd:c * hq * hd + (j + 1) * hd],
                        h_c, skip_group_check=True, **fl,
                    )
                nc.tensor.matmul(
                    kT_ps[:], wkv_sb[:, c * 2 * hd:c * 2 * hd + hd], h_c,
                    skip_group_check=True, **fl,
                )
                nc.tensor.matmul(
                    v_ps[:], h_c, wkv_sb[:, c * 2 * hd + hd:(c + 1) * 2 * hd],
                    skip_group_check=True, **fl,
                )

            # ---- RoPE ----
            def rope_parts(src, cosv, sinv, nsinv, n):
                t1f = work_p.tile([128, jt], F32, tag="r1", name="r1")
                t1 = t1f[:, 0:n]
                nc.vector.tensor_mul(t1, src, cosv)
                t2f = work_p.tile([128, jt], F32, tag="r2", name="r2")
                t2 = t2f[:, 0:n]
                nc.vector.tensor_mul(t2[0:half, :], src[half:hd, :], nsinv[0:half, :])
                nc.vector.tensor_mul(t2[half:hd, :], src[0:half, :], sinv[half:hd, :])
                return t1, t2

            # qT_sb layout: [d, (b, j, t)] so each batch slice is contiguous;
            # the rope add scatters from the projection's (j, b, t) order.
            qT_sb = work_p.tile([128, jt], F32, tag="qT")
            t1, t2 = rope_parts(qT_ps[:], cos4, sin4, nsin4, jt)
            qdst = qT_sb[:].rearrange("p (bb j t) -> p j bb t", bb=b, j=hq)
            nc.vector.tensor_add(
                qdst,
                t1.rearrange("p (j bb t) -> p j bb t", j=hq, bb=b),
                t2.rearrange("p (j bb t) -> p j bb t", j=hq, bb=b),
            )
            qT_bf = work_p.tile([128, jt], BF16, tag="qTbf")
            nc.vector.tensor_copy(qT_bf[:], qT_sb[:])

            kT_new = work_p.tile([128, tok], F32, tag="kTn")
            t1k, t2k = rope_parts(kT_ps[:], cos4[:, 0:tok], sin4[:, 0:tok],
                                  nsin4[:, 0:tok], tok)
            nc.vector.tensor_add(kT_new[:], t1k, t2k)

            # fresh v: copy out of psum, then PE row-shift each batch slice to
            # partition base 0 (stationary operand base must be 0/32/64/96)
            v_sb = work_p.tile([tok, hd], F32, tag="vsb")
            nc.scalar.copy(v_sb[:], v_ps[:])
            v_new = []
            for bb in range(b):
                sh_ps = ps.tile([s, hd], F32, tag="kv", name=f"vsh{bb}")
                nc.tensor.matmul(
                    sh_ps[:], isel[:, bb * s:(bb + 1) * s], v_sb[:],
                    start=True, stop=True,
                )
                vn = work_p.tile([s, hd], F32, tag=f"vn{bb}", name=f"vn{bb}")
                nc.scalar.copy(vn[:], sh_ps[:])
                v_new.append(vn)

            # ---- attention (denominator via PE) ----
            # den_ps[1, (j,t)] accumulates column sums of every exp tile plus
            # the masked fresh exp; it needs no v, so recip/broadcast complete
            # during the kv transfers. The den matmuls are issued AFTER the
            # whole score/exp chain so the in-order PE never stalls on Act.
            attnT_sb = work_p.tile([128, jt], BF16, tag="attnT")  # (j, b, t)
            outT_sb = work_p.tile([128, MC * tok], BF16, tag="outT")  # (mc,b,t)
            for bb in range(b):
                qT_b = qT_bf[:, bb * rows:(bb + 1) * rows]
                qT_b32 = qT_sb[:, bb * rows:(bb + 1) * rows]
                kt = kts[bb]
                vt = vts[bb]
                exs = []
                for g in range(ktiles // GRP):
                    sc_ps = ps.tile([128, GRP * rows], F32, tag="sc",
                                    name=f"sc{bb}_{g}")
                    for u in range(GRP):
                        tt = g * GRP + u
                        nc.tensor.matmul(
                            sc_ps[:, u * rows:(u + 1) * rows],
                            kt[:, tt * 128:(tt + 1) * 128], qT_b,
                            start=(u == 0), stop=(u == GRP - 1),
                        )
                    ex = exp_p.tile([128, GRP * rows], BF16, tag="ex",
                                    name=f"ex{bb}_{g}", bufs=10)
                    nc.scalar.activation(ex[:], sc_ps[:], AF.Exp)
                    exs.append(ex)
                # fresh keys (the only masked block)
                scn_ps = ps.tile([s, rows], F32, tag="kv", name="scn")
                nc.tensor.matmul(
                    scn_ps[:], kT_new[:, bb * s:(bb + 1) * s], qT_b32,
                    start=True, stop=True,
                )
                exn = exp_p.tile([s, rows], F32, tag="exn")
                nc.scalar.activation(exn[:], scn_ps[:], AF.Exp)
                nc.vector.tensor_mul(exn[:], exn[:], maskT)
                # denominator accumulation (PE, reads only exp tiles)
                den_ps = ps.tile([1, rows], F32, tag="kv", name="den")
                for g in range(ktiles // GRP):
                    for u in range(GRP):
                        nc.tensor.matmul(
                            den_ps[:], ones_bf[:],
                            exs[g][:, u * rows:(u + 1) * rows],
                            start=(g == 0 and u == 0), stop=False,
                            skip_group_check=True,
                        )
                nc.tensor.matmul(
                    den_ps[:], ones_col[0:s, :], exn[:],
                    start=False, stop=True, skip_group_check=True,
                )
                rden = acc_p.tile([1, rows], F32, tag="rden")
                nc.vector.reciprocal(rden[:], den_ps[:])
                bc_ps = ps.tile([128, rows], F32, tag="kv", name="bc")
                nc.tensor.matmul(bc_ps[:], ones_row[:], rden[:],
                                 start=True, stop=True)
                rdenb = acc_p.tile([128, rows], F32, tag="rdenb")
                nc.scalar.copy(rdenb[:], bc_ps[:])
                # attn-v accumulation + fresh, then normalize into (j, b, t)
                attn_ps = ps.tile([128, rows], F32, tag="attn",
                                  name=f"attn{bb}")
                for tt in range(ktiles):
                    nc.tensor.matmul(
                        attn_ps[:], vt[:, tt * hd:(tt + 1) * hd],
                        exs[tt // GRP][:, (tt % GRP) * rows:
                                       (tt % GRP + 1) * rows],
                        start=(tt == 0), stop=False, skip_group_check=True,
                    )
                nc.tensor.matmul(
                    attn_ps[:], v_new[bb][:], exn[:],
                    start=False, stop=True, skip_group_check=True,
                )
                adst = attnT_sb[:].rearrange(
                    "p (j bb t) -> p j bb t", j=hq, bb=b)[:, :, bb, :]
                nc.vector.tensor_mul(
                    adst,
                    attn_ps[:].rearrange("p (j t) -> p j t", j=hq),
                    rdenb[:].rearrange("p (j t) -> p j t", j=hq),
                )

            # ---- o_proj, chasing the wo chunk arrivals ----
            # outT[m, (mc, b, t)] = sum_j wo_j[:, mc]^T @ attnT_j  (all 64
            # (b,t) columns per matmul; chunks are disjoint mc column groups)
            for (wog, off, mcg) in wo_tiles:
                oTg = ps.tile([128, mcg * tok], F32, tag="oT",
                              name=f"oT{off}")
                for m in range(mcg):
                    for j in range(hq):
                        nc.tensor.matmul(
                            oTg[:, m * tok:(m + 1) * tok],
                            wog[:, j * mcg * 128 + m * 128:
                                j * mcg * 128 + (m + 1) * 128],
                            attnT_sb[:, j * tok:(j + 1) * tok],
                            start=(j == 0), stop=(j == hq - 1),
                            skip_group_check=True,
                        )
                nc.vector.tensor_copy(
                    outT_sb[:, off * tok:(off + mcg) * tok], oTg[:])

            nc.sync.dma_start(out_d[:], outT_sb[:])

    nc.compile()
    return nc


_NC_CACHE = {}


def _get_nc(key=(B, S, H, HQ, HD, PAST)):
    if key not in _NC_CACHE:
        _NC_CACHE[key] = build_nc(*key)
    return _NC_CACHE[key]


def make_in_maps(hidden_states, k_cache, v_cache, Wq, Wk, Wv, Wo, position_ids):
    """Host-side shard + layout prep: one input dict per core."""
    bf = ml_dtypes.bfloat16
    hid = hidden_states.reshape(TOK, H).astype(np.float32)
    # hT[p, c*TOK + t] = hidden[t, c*128+p]
    hT = np.ascontiguousarray(
        hid.T.reshape(NCH, 128, TOK).transpose(1, 0, 2).reshape(128, NCH * TOK)
    ).astype(bf)
    # RoPE tables, tiled 4x over j: [128, (j, b, t)]
    inv_freq = 1.0 / (ROPE_BASE ** (np.arange(0, HD, 2, dtype=np.float64) / HD))
    ang = position_ids.astype(np.float64).reshape(-1)[None, :] * np.concatenate(
        [inv_freq, inv_freq])[:, None]                      # [hd, tok]
    cosT = np.cos(ang).astype(np.float32)
    sinT = np.sin(ang).astype(np.float32)
    cos4 = np.tile(cosT, (1, HQ))
    sin4 = np.tile(sinT, (1, HQ))
    nsin4 = -sin4
    # mask over fresh keys: maskT[j, (h, t)] = 1 if j <= t
    jj = np.arange(S)[:, None]
    tt = np.tile(np.arange(S)[None, :], (1, HQ)).reshape(1, ROWS)
    maskkT = np.zeros((128, TOK), np.float32)
    maskkT[0:S, :] = (jj <= tt).astype(np.float32)
    consts = np.ascontiguousarray(
        np.concatenate([cos4, sin4, nsin4, maskkT], axis=1))

    in_maps = []
    for c in range(NCORES):
        q0 = c * HQ * HD
        # wq[p, (c, j, d)] = Wq[c*128+p, q0 + j*128 + d] * SCALE
        wq = np.ascontiguousarray(
            (Wq[:, q0:q0 + HQ * HD] * SCALE).astype(np.float32)
            .reshape(NCH, 128, HQ * HD).transpose(1, 0, 2).reshape(128, -1)
        ).astype(bf)
        # wkv[p, (c, {k:0,v:1}, d)]
        wkv = np.concatenate(
            [Wk[:, c * HD:(c + 1) * HD].reshape(NCH, 128, HD),
             Wv[:, c * HD:(c + 1) * HD].reshape(NCH, 128, HD)], axis=2
        ).astype(np.float32).transpose(1, 0, 2).reshape(128, NCH * 2 * HD)
        wkv = np.ascontiguousarray(wkv).astype(bf)
        # wo[p, (g, j, mc in g, m)] = Wo[q0 + j*128 + p, mc*128 + m]
        wo4 = (Wo[q0:q0 + HQ * HD, :].astype(np.float32)
               .reshape(HQ, 128, MC, 128))          # [j, p, mc, m]
        blocks = []
        off = 0
        for mcg in [5, 5, 5, 5, 4, 4, 2, 2]:
            blk = wo4[:, :, off:off + mcg, :].transpose(1, 0, 2, 3)
            blocks.append(blk.reshape(128, HQ * mcg * 128))
            off += mcg
        wo = np.ascontiguousarray(np.concatenate(blocks, axis=1)).astype(bf)
        in_maps.append({
            "consts": consts,
            "hT": hT,
            "wq": wq,
            "wkv": wkv,
            "wo": wo,
            "kT": np.ascontiguousarray(
                k_cache[:, :, c, :].transpose(0, 2, 1)).astype(bf),
            # v_r[b, p, tt*HD+d] = v[b, tt*128+p, d]
            "v": np.ascontiguousarray(
                v_cache[:, :, c, :].reshape(B, PAST // 128, 128, HD)
                .transpose(0, 2, 1, 3).reshape(B, 128, PAST)
            ).astype(bf),
        })
    return in_maps


def kernel(hidden_states, k_cache, v_cache, Wq, Wk, Wv, Wo, position_ids):
    nc = _get_nc()
    in_maps = make_in_maps(
        np.asarray(hidden_states), np.asarray(k_cache), np.asarray(v_cache),
        np.asarray(Wq), np.asarray(Wk), np.asarray(Wv), np.asarray(Wo),
        np.asarray(position_ids),
    )
    res = run_bass_kernel_spmd(nc, in_maps, list(range(NCORES)))
    # out_p[p, (mc, b, t)] -> out[(b,t), mc*128+p]; host sums the 8 partials
    acc = np.zeros((128, MC * TOK), np.float32)
    for c in range(NCORES):
        acc += res.results[c]["out_p"].astype(np.float32)
    out = acc.reshape(128, MC, TOK).transpose(2, 1, 0).reshape(TOK, H)
    return np.ascontiguousarray(out).reshape(B, S, H)
